# revision 1
# baseline (speedup 1.0000x reference)
"""FFM CrossLayer pairwise-interaction kernel for 8x Trainium2 NeuronCores.

Math: out[b] = sum_{i<j} <K[i,f_j,:], K[j,f_i,:]> * x[b,i] * x[b,j]
With W[i,j] = sum_o K[i,f_j,o]*K[j,f_i,o] (symmetric), this equals
    out[b] = 0.5 * (x_b^T W x_b - sum_i W[i,i] * x[b,i]^2).

Strategy (8 cores):
  - W build is o-sharded: core c computes the partial W over output-dims
    o in [8c, 8c+8) plus the partial diagonal, then a 1MB AllReduce(add)
    yields the full W (and diag row) on every core.
      term1[j,(i,o)] = K[i,f_j,o] via one-hot matmul on PE
      term2[j,(i,o)] = K[j,f_i,o] via gpsimd ap_gather (per-partition
                       own-row gather, shared index pattern)
      Z = term1*term2 on DVE, then grouped reduce over o.
  - Main compute is batch-sharded: core c computes y = x_c @ W for its 512
    batch rows (PE), then out = 0.5*sum_i y*x - sum_i (0.5*d_i)*x^2 via a
    fused DVE tensor_tensor_reduce plus a tiny N=1 matmul for the diag term.
"""

import sys

import numpy as np

try:  # the grading env may or may not have concourse on sys.path already
    import concourse.bass as bass  # noqa: F401
except ImportError:
    sys.path.insert(0, "/opt/trn_rl_repo")

import concourse.bacc as bacc
import concourse.bass as bass
import concourse.mybir as mybir
import concourse.tile as tile
from concourse.bass_utils import run_bass_kernel_spmd

B, D, F, O = 4096, 512, 64, 64
NC = 8            # cores
OS = O // NC      # o-slice per core (8)
BS = B // NC      # batch shard per core (512)
P = 128           # partitions
NJT = D // P      # j tiles (4)
F32 = mybir.dt.float32
F32R = mybir.dt.float32r

_CACHE = {}


def _r(ap):
    return ap  # f32 for now; f32r requires a rounding producer op


def _build_program(collective=True):
    nc = bacc.Bacc("TRN2", target_bir_lowering=False, debug=False, num_devices=NC)

    ehot_fj = nc.dram_tensor("ehot_fj", [F, D], F32, kind="ExternalInput").ap()
    ehot_if = nc.dram_tensor("ehot_if", [D, F], F32, kind="ExternalInput").ap()
    kt = nc.dram_tensor("kt", [F, D * OS], F32, kind="ExternalInput").ap()
    ko = nc.dram_tensor("ko", [D, F * OS], F32, kind="ExternalInput").ap()
    idx = nc.dram_tensor("idx", [P, D // 16], mybir.dt.int16, kind="ExternalInput").ap()
    xT = nc.dram_tensor("xT", [D, BS], F32, kind="ExternalInput").ap()
    xc = nc.dram_tensor("xc", [BS, D], F32, kind="ExternalInput").ap()
    outv = nc.dram_tensor("outv", [BS], F32, kind="ExternalOutput").ap()

    with tile.TileContext(nc) as tc:
        with (
            tc.tile_pool(name="cst", bufs=1) as cst,
            tc.tile_pool(name="sb", bufs=2) as sb,
            tc.tile_pool(name="wpool", bufs=1) as wpool,
            tc.tile_pool(name="psA", bufs=2, space="PSUM") as psA,
            tc.tile_pool(name="psY", bufs=2, space="PSUM") as psY,
            tc.tile_pool(name="psD", bufs=2, space="PSUM") as psD,
            tc.tile_pool(name="dram", bufs=1, space="DRAM") as dram,
        ):
            # ---- constant loads ----
            ehot_fj_sb = cst.tile([F, D], F32)
            nc.sync.dma_start(ehot_fj_sb[:], ehot_fj[:])
            eh_r = cst.tile([F, D], F32R, tag="eh_r")
            nc.scalar.copy(eh_r[:], ehot_fj_sb[:])
            kt_sb = cst.tile([F, D * OS], F32)
            nc.sync.dma_start(kt_sb[:], kt[:])
            kt_r = cst.tile([F, D * OS], F32R, tag="kt_r")
            nc.scalar.copy(kt_r[:], kt_sb[:])
            idx_sb = cst.tile([P, D // 16], mybir.dt.int16)
            nc.sync.dma_start(idx_sb[:], idx[:])
            ko_sb = []
            for jt in range(NJT):
                t = cst.tile([P, F * OS], F32, tag=f"ko{jt}")
                nc.sync.dma_start(t[:], ko[jt * P : (jt + 1) * P, :])
                ko_sb.append(t)
            eif_sb = []
            for it in range(NJT):
                t = cst.tile([P, F], F32, tag=f"eif{it}")
                nc.sync.dma_start(t[:], ehot_if[it * P : (it + 1) * P, :])
                eif_sb.append(t)

            wpd = dram.tile([D + 1, D], F32)   # rows 0..511 = partial W, row 512 = partial 0.5*diag
            wrd = dram.tile([D + 1, D], F32)

            # ---- phase A: partial W (o-slice) ----
            wp_sb = []
            for jt in range(NJT):
                w_t = wpool.tile([P, D], F32, tag=f"wp{jt}")
                wp_sb.append(w_t)
                t2 = sb.tile([P, D * OS], F32, tag="t2")
                nc.gpsimd.ap_gather(
                    t2[:], ko_sb[jt][:], idx_sb[:],
                    channels=P, num_elems=F, d=OS, num_idxs=D,
                )
                for q in range(4):  # quarters of the (i,o) axis: 128 i each
                    pt = psA.tile([P, P * OS], F32, tag="pt")  # [128,1024]
                    for n in range(2):
                        nc.tensor.matmul(
                            pt[:, n * 512 : (n + 1) * 512],
                            eh_r[:, jt * P : (jt + 1) * P],
                            kt_r[:, q * P * OS + n * 512 : q * P * OS + (n + 1) * 512],
                            start=True, stop=True,
                        )
                    z = sb.tile([P, P * OS], F32, tag="z")
                    nc.vector.tensor_mul(z[:], pt[:], t2[:, q * P * OS : (q + 1) * P * OS])
                    zv = z[:].rearrange("p (i o) -> p i o", o=OS)
                    nc.vector.tensor_reduce(
                        w_t[:, q * P : (q + 1) * P], zv,
                        axis=mybir.AxisListType.X, op=mybir.AluOpType.add,
                    )
                nc.sync.dma_start(wpd[jt * P : (jt + 1) * P, :], w_t[:])

            # ---- partial diagonal: 0.5 * sum_{o in slice} K[i, f_i, o]^2 ----
            for it in range(NJT):
                sq = sb.tile([P, F * OS], F32, tag="sq")
                nc.vector.tensor_mul(sq[:], ko_sb[it][:], ko_sb[it][:])
                sqr = sb.tile([P, F], F32, tag="sqr")
                nc.vector.tensor_reduce(
                    sqr[:], sq[:].rearrange("p (f o) -> p f o", o=OS),
                    axis=mybir.AxisListType.X, op=mybir.AluOpType.add,
                )
                junk = sb.tile([P, F], F32, tag="junkd")
                nc.vector.tensor_mul(junk[:], sqr[:], eif_sb[it][:])
                dcol = sb.tile([P, 1], F32, tag="dcol")
                nc.vector.tensor_reduce(
                    dcol[:], junk[:], axis=mybir.AxisListType.X, op=mybir.AluOpType.add,
                )
                nc.sync.dma_start(wpd[D : D + 1, it * P : (it + 1) * P], dcol[:])

            # ---- AllReduce partial W + diag ----
            if collective:
                nc.gpsimd.collective_compute(
                    "AllReduce", mybir.AluOpType.add,
                    replica_groups=[list(range(NC))],
                    ins=[wpd.opt()], outs=[wrd.opt()],
                )
            else:  # timing-sim variant: plain copy stands in for the collective
                nc.sync.dma_start(wrd[:], wpd[:])

            # ---- phase B: y = x_c @ W, epilogue ----
            xT_sb = []
            xsq_sb = []
            for jc in range(NJT):
                t0_ = cst.tile([P, BS], F32, tag=f"xTf{jc}")
                nc.sync.dma_start(t0_[:], xT[jc * P : (jc + 1) * P, :])
                t = cst.tile([P, BS], F32R, tag=f"xT{jc}")
                nc.scalar.copy(t[:], t0_[:])
                xT_sb.append(t)
                tq = cst.tile([P, BS], F32, tag=f"xsq{jc}")
                nc.vector.tensor_mul(tq[:], t0_[:], t0_[:])
                xsq_sb.append(tq)
            xc_sb = []
            for bt in range(NJT):
                t = cst.tile([P, D], F32, tag=f"xc{bt}")
                nc.sync.dma_start(t[:], xc[bt * P : (bt + 1) * P, :])
                xc_sb.append(t)
            w_sb = []
            dcol_sb = []
            for jc in range(NJT):
                tf_ = wpool.tile([P, D], F32, tag=f"wf{jc}")
                nc.sync.dma_start(tf_[:], wrd[jc * P : (jc + 1) * P, :])
                t = wpool.tile([P, D], F32R, tag=f"w{jc}")
                nc.scalar.copy(t[:], tf_[:])
                w_sb.append(t)
                dt_ = wpool.tile([P, 1], F32, tag=f"d{jc}")
                nc.sync.dma_start(dt_[:], wrd[D : D + 1, jc * P : (jc + 1) * P])
                dcol_sb.append(dt_)

            for bt in range(NJT):
                yp = psY.tile([P, D], F32, tag="yp")
                for jc in range(NJT):
                    nc.tensor.matmul(
                        yp[:], xT_sb[jc][:, bt * P : (bt + 1) * P], w_sb[jc][:],
                        start=(jc == 0), stop=(jc == NJT - 1),
                    )
                y2p = psD.tile([P, 1], F32, tag="y2p")
                for it in range(NJT):
                    nc.tensor.matmul(
                        y2p[:], xsq_sb[it][:, bt * P : (bt + 1) * P], dcol_sb[it][:],
                        start=(it == 0), stop=(it == NJT - 1),
                    )
                junk2 = sb.tile([P, D], F32, tag="junk2")
                nc.vector.tensor_mul(junk2[:], yp[:], xc_sb[bt][:])
                sres = sb.tile([P, 1], F32, tag="sres")
                nc.vector.tensor_reduce(
                    sres[:], junk2[:], axis=mybir.AxisListType.X, op=mybir.AluOpType.add,
                )
                ov = sb.tile([P, 1], F32, tag="ov")
                nc.vector.tensor_sub(ov[:], sres[:], y2p[:])
                ovh = sb.tile([P, 1], F32, tag="ovh")
                nc.scalar.mul(ovh[:], ov[:], 0.5)
                nc.sync.dma_start(outv[bt * P : (bt + 1) * P], ovh[:])

    nc.compile()
    return nc


def _host_prep(x, kern, field_ids):
    x = np.ascontiguousarray(np.asarray(x, dtype=np.float32))
    k = np.ascontiguousarray(np.asarray(kern, dtype=np.float32))
    fid = np.asarray(field_ids).astype(np.int64).ravel()
    assert x.shape == (B, D) and k.shape == (D, F, O) and fid.shape == (D,)

    ehot_fj = (fid[None, :] == np.arange(F)[:, None]).astype(np.float32)  # [F, D]
    ehot_if = np.ascontiguousarray(ehot_fj.T)                              # [D, F]
    idx16 = np.zeros((16, D // 16), np.int16)
    for kk in range(D):
        idx16[kk % 16, kk // 16] = fid[kk]
    idx_w = np.tile(idx16, (P // 16, 1))

    in_maps = []
    for c in range(NC):
        ksl = k[:, :, c * OS : (c + 1) * OS]                   # [D, F, OS]
        kt_c = np.ascontiguousarray(ksl.transpose(1, 0, 2)).reshape(F, D * OS)
        ko_c = np.ascontiguousarray(ksl).reshape(D, F * OS)
        xs = x[c * BS : (c + 1) * BS]
        in_maps.append({
            "ehot_fj": ehot_fj, "ehot_if": ehot_if,
            "kt": kt_c, "ko": ko_c, "idx": idx_w,
            "xT": np.ascontiguousarray(xs.T), "xc": xs,
        })
    return in_maps


def kernel(x, kernel, field_ids):
    if "nc" not in _CACHE:
        _CACHE["nc"] = _build_program()
    nc = _CACHE["nc"]
    in_maps = _host_prep(x, kernel, field_ids)
    res = run_bass_kernel_spmd(nc, in_maps, core_ids=list(range(NC)))
    out = np.concatenate([np.asarray(res.results[c]["outv"]).ravel() for c in range(NC)])
    return out.astype(np.float32)



# revision 3
# speedup vs baseline: 1.8947x; 1.8947x over previous
"""FFM CrossLayer pairwise-interaction kernel for 8x Trainium2 NeuronCores.

Math: out[b] = sum_{i<j} <K[i,f_j,:], K[j,f_i,:]> * x[b,i] * x[b,j]
With W[i,j] = sum_o K[i,f_j,o]*K[j,f_i,o] (symmetric), this equals
    out[b] = 0.5 * (x_b^T W x_b - sum_i W[i,i] * x[b,i]^2).

Strategy (v2, stripe-sharded, no W collective):
  Core c owns the column stripe J_c = [64c, 64c+64) of W.
  Phase A (local): W[:, J_c] via
      term1[i,(j,o)] = K[i, f_j, o]   (gpsimd ap_gather from G=[D, F*O] bf16)
      term2[i,(j,o)] = K[j, f_i, o]   (PE one-hot matmul: E_if^T @ R_c)
      W_ib = reduce_o(term1 * term2)  (scalar psum->bf16 copy, DVE mul +
                                       grouped reduce), [128, 64] bf16 per
                                       i-block; cols 64..127 of the lhsT get
                                       W_ib * dmask_ib (diag correction).
  Phase B (full batch per core): per 512-batch chunk,
      psum[0:64,  b] = sum_i W[i, j] xT[i, b]      (main term)
      psum[64:128,b] = -W[jj, jj] xT[jj, b]        (diag term, via dmask cols)
      z = psum * xs2 (x stripe stacked twice), out_part = matmul(0.5-ones, z).
  Collective: ReduceScatter(add) of the 8 per-core [4096] partials -> [512].
"""

import sys

import numpy as np

try:  # the grading env may or may not have concourse on sys.path already
    import concourse.bass as bass  # noqa: F401
except ImportError:
    sys.path.insert(0, "/opt/trn_rl_repo")

import ml_dtypes

import concourse.bacc as bacc
import concourse.bass as bass
import concourse.mybir as mybir
import concourse.tile as tile
from concourse.bass_utils import run_bass_kernel_spmd

B, D, F, O = 4096, 512, 64, 64
NC = 8            # cores
JS = D // NC      # stripe width (64)
P = 128           # partitions
NIB = D // P      # i blocks (4)
NBC = B // 512    # batch chunks (8)
F32 = mybir.dt.float32
BF16 = mybir.dt.bfloat16
NPBF16 = ml_dtypes.bfloat16

_CACHE = {}


def _build_program():
    nc = bacc.Bacc("TRN2", target_bir_lowering=False, debug=False, num_devices=NC)

    g = nc.dram_tensor("g", [D, F * O], BF16, kind="ExternalInput").ap()
    rc = nc.dram_tensor("rc", [F, JS * O], BF16, kind="ExternalInput").ap()
    eif = nc.dram_tensor("eif", [F, D], BF16, kind="ExternalInput").ap()
    idx = nc.dram_tensor("idx", [P, JS // 16], mybir.dt.int16, kind="ExternalInput").ap()
    xt = nc.dram_tensor("xt", [D, B], BF16, kind="ExternalInput").ap()
    xs2 = nc.dram_tensor("xs2", [P, B], BF16, kind="ExternalInput").ap()
    dmask = nc.dram_tensor("dmask", [D, JS], BF16, kind="ExternalInput").ap()
    hov = nc.dram_tensor("hov", [P, 1], BF16, kind="ExternalInput").ap()
    outv = nc.dram_tensor("outv", [B // NC], F32, kind="ExternalOutput").ap()

    FO = F * O  # 4096

    with tile.TileContext(nc) as tc:
        with (
            tc.tile_pool(name="cst", bufs=1) as cst,
            tc.tile_pool(name="sbA", bufs=2) as sbA,
            tc.tile_pool(name="sbB", bufs=2) as sbB,
            tc.tile_pool(name="wp", bufs=1) as wp,
            tc.tile_pool(name="psA", bufs=2, space="PSUM") as psA,
            tc.tile_pool(name="psB", bufs=2, space="PSUM") as psB,
            tc.tile_pool(name="psO", bufs=2, space="PSUM") as psO,
            tc.tile_pool(name="dram", bufs=1, space="DRAM") as dram,
        ):
            # ---- constant loads (phase A deps first) ----
            idx_sb = cst.tile([P, JS // 16], mybir.dt.int16)
            nc.sync.dma_start(idx_sb[:], idx[:])
            rc_sb = cst.tile([F, JS * O], BF16)
            nc.sync.dma_start(rc_sb[:], rc[:])
            eif_sb = cst.tile([F, D], BF16)
            nc.sync.dma_start(eif_sb[:], eif[:])
            g_sb = cst.tile([P, NIB * FO], BF16)
            nc.sync.dma_start(
                g_sb[:].rearrange("p (a n) -> p a n", a=NIB),
                g[:].rearrange("(a p) n -> p a n", p=P),
            )
            # phase B deps
            dm_sb = cst.tile([P, NIB * JS], BF16)
            nc.sync.dma_start(
                dm_sb[:].rearrange("p (a j) -> p a j", a=NIB),
                dmask[:].rearrange("(a p) j -> p a j", p=P),
            )
            hov_sb = cst.tile([P, 1], BF16)
            nc.sync.dma_start(hov_sb[:], hov[:])
            xs2_sb = cst.tile([P, B], BF16)
            nc.sync.dma_start(xs2_sb[:], xs2[:])
            xt_sb = cst.tile([P, NIB * B], BF16)
            nc.sync.dma_start(
                xt_sb[:].rearrange("p (a n) -> p a n", a=NIB),
                xt[:].rearrange("(a p) n -> p a n", p=P),
            )

            ovrow = cst.tile([1, B], F32)
            rsin = dram.tile([B], F32)
            rsout = dram.tile([B // NC], F32)

            wlhs = []
            for ib in range(NIB):
                w_t = wp.tile([P, P], BF16, tag=f"wl{ib}")
                wlhs.append(w_t)

            # ---- phase A: W[:, J_c] stripe, one i-block at a time ----
            with nc.allow_low_precision(reason="bf16 W stripe; tol 2e-2"):
                for ib in range(NIB):
                    t1 = sbA.tile([P, FO], BF16, tag="t1")
                    nc.gpsimd.ap_gather(
                        t1[:], g_sb[:, ib * FO : (ib + 1) * FO], idx_sb[:],
                        channels=P, num_elems=F, d=O, num_idxs=JS,
                    )
                    for q in range(4):  # 1024 cols = 16 j x 64 o
                        ps = psA.tile([P, 1024], F32, tag="psA")
                        for n in range(2):
                            nc.tensor.matmul(
                                ps[:, n * 512 : (n + 1) * 512],
                                eif_sb[:, ib * P : (ib + 1) * P],
                                rc_sb[:, q * 1024 + n * 512 : q * 1024 + (n + 1) * 512],
                                start=True, stop=True,
                            )
                        t2b = sbA.tile([P, 1024], BF16, tag="t2b")
                        nc.scalar.copy(t2b[:], ps[:])
                        zb = sbA.tile([P, 1024], BF16, tag="zb")
                        nc.vector.tensor_mul(
                            zb[:], t1[:, q * 1024 : (q + 1) * 1024], t2b[:]
                        )
                        nc.vector.tensor_reduce(
                            wlhs[ib][:, q * 16 : q * 16 + 16],
                            zb[:].rearrange("p (j o) -> p j o", o=O),
                            axis=mybir.AxisListType.X, op=mybir.AluOpType.add,
                        )
                    # diag-correction columns 64..127 of the lhsT
                    nc.vector.tensor_mul(
                        wlhs[ib][:, JS : 2 * JS],
                        wlhs[ib][:, 0:JS],
                        dm_sb[:, ib * JS : (ib + 1) * JS],
                    )

            # ---- phase B: y^T = lhsT^T @ xT over full batch, fused epilogue ----
            for bc in range(NBC):
                yp = psB.tile([P, 512], F32, tag="yp")
                for ib in range(NIB):
                    nc.tensor.matmul(
                        yp[:],
                        wlhs[ib][:],
                        xt_sb[:, ib * B + bc * 512 : ib * B + (bc + 1) * 512],
                        start=(ib == 0), stop=(ib == NIB - 1),
                    )
                z = sbB.tile([P, 512], BF16, tag="z")
                nc.vector.tensor_mul(z[:], yp[:], xs2_sb[:, bc * 512 : (bc + 1) * 512])
                op = psO.tile([1, 512], F32, tag="op")
                nc.tensor.matmul(op[:], hov_sb[:], z[:], start=True, stop=True)
                nc.scalar.copy(ovrow[:, bc * 512 : (bc + 1) * 512], op[:])

            # ---- collective: tiny ReduceScatter of the partial outputs ----
            nc.sync.dma_start(rsin[:], ovrow[:])
            nc.gpsimd.collective_compute(
                "ReduceScatter", mybir.AluOpType.add,
                replica_groups=[list(range(NC))],
                ins=[rsin.opt()], outs=[rsout.opt()],
            )
            nc.sync.dma_start(outv[:], rsout[:])

    nc.compile()
    return nc


def _host_prep(x, kern, field_ids):
    x = np.asarray(x, dtype=np.float32)
    k = np.asarray(kern, dtype=np.float32)
    fid = np.asarray(field_ids).astype(np.int64).ravel()
    assert x.shape == (B, D) and k.shape == (D, F, O) and fid.shape == (D,)

    g = np.ascontiguousarray(k.reshape(D, F * O)).astype(NPBF16)
    eif = (fid[None, :] == np.arange(F)[:, None]).astype(NPBF16)  # [F, D]
    xt = np.ascontiguousarray(x.T).astype(NPBF16)                  # [D, B]
    hov = np.full((P, 1), 0.5, NPBF16)

    in_maps = []
    for c in range(NC):
        jlo = c * JS
        jc = slice(jlo, jlo + JS)
        rc = np.ascontiguousarray(k[jc].transpose(1, 0, 2).reshape(F, JS * O)).astype(NPBF16)
        fj = fid[jc].astype(np.int16)
        idx16 = np.zeros((16, JS // 16), np.int16)
        for t in range(JS):
            idx16[t % 16, t // 16] = fj[t]
        idx = np.tile(idx16, (P // 16, 1))
        xsl = xt[jc]                                   # [64, B] bf16
        xs2 = np.concatenate([xsl, xsl], axis=0)       # [128, B]
        dmask = np.zeros((D, JS), NPBF16)
        for t in range(JS):
            dmask[jlo + t, t] = -1.0
        in_maps.append({
            "g": g, "rc": rc, "eif": eif, "idx": idx,
            "xt": xt, "xs2": xs2, "dmask": dmask, "hov": hov,
        })
    return in_maps


def kernel(x, kernel, field_ids):
    if "nc" not in _CACHE:
        _CACHE["nc"] = _build_program()
    nc = _CACHE["nc"]
    in_maps = _host_prep(x, kernel, field_ids)
    res = run_bass_kernel_spmd(nc, in_maps, core_ids=list(range(NC)))
    out = np.concatenate([np.asarray(res.results[c]["outv"]).ravel() for c in range(NC)])
    return out.astype(np.float32)
